# revision 5
# baseline (speedup 1.0000x reference)
"""Trainium2 Bass kernel for nn_BaselineGAT (LayerNorm + MLP + GATConv).

Two SPMD phases over 8 NeuronCores, nodes sharded naturally (core c owns
rows [c*6272, (c+1)*6272)):

Phase 1 (per core): LayerNorm (stats via ones-matmul in fp32 PSUM, fused
  scale into x) + MLP 1488->1024->512 with fp16 matmul inputs, then
  row-major heads producing the fp16 message table row
  [g (256, c-major) | a_src (8) | pad] (768B) and fp32 [res | a_dst] rows.

Host: concat table shards -> full table, split at 32768 rows (int16
  gather index limit).

Phase 2 (per core, dense slot-major edge layout): edges grouped by
  dst-window (128 dsts), within a window pass-A slots (src<32768) then
  pass-B, dst-major dense, padded to 128-slot chunks. Per chunk:
  dma_gather of 128 table rows (by src) + 128 a_dst rows (by local dst),
  e = lrelu(a_src+a_dst), ex = exp(e), msg = [g*ex_bcast | ex], then a
  matmul with a host-built 0/1 slot->dst matrix accumulates
  [agg | den] into the window PSUM (the per-destination softmax
  scatter-add runs on the PE). Window epilogue: out = elu(agg/(den+eps)
  + bg) + res, written straight to DRAM (no phase 3).
"""

import sys

sys.path.insert(0, "/opt/trn_rl_repo")

from dataclasses import dataclass

import numpy as np

import concourse.bass as bass  # noqa: F401
import concourse.mybir as mybir
import concourse.tile as tile
from concourse import bacc
from concourse.bass_utils import run_bass_kernel_spmd
from concourse.library_config import mlp as mlp_lib

P = 128
F32 = mybir.dt.float32
F16 = mybir.dt.float16
I16 = mybir.dt.int16
AL = mybir.AluOpType
AF = mybir.ActivationFunctionType


@dataclass(frozen=True)
class Cfg:
    n_nodes: int = 50000
    n_edges: int = 800000
    d_in: int = 1488
    d_in_pad: int = 1536
    d_hid: int = 1024
    d_out: int = 512
    C: int = 32
    H: int = 8
    n_cores: int = 8
    node_chunk: int = 512
    split: int = 32768
    row_w: int = 384          # fp16 elems per table row (768B)

    @property
    def d_head(self):
        return self.C * self.H  # 256

    @property
    def w3(self):
        return self.d_head + self.H  # 264

    @property
    def rows_per_core(self):
        return 6272

    @property
    def table_rows(self):
        return self.rows_per_core * self.n_cores  # 50176

    @property
    def n_win(self):
        return self.rows_per_core // P  # 49


CFG = Cfg()

_NC_CACHE = {}


def wrap_idx(lst: np.ndarray) -> np.ndarray:
    n = len(lst)
    assert n % 16 == 0
    lay = lst.reshape(n // 16, 16).T.copy()
    return np.tile(lay, (8, 1)).astype(np.int16)


# ----------------------------------------------------------------------------
# Edge plan (host)
# ----------------------------------------------------------------------------

def build_plan(cfg: Cfg, edge_index):
    N, R, NWIN, SPLIT = cfg.n_nodes, cfg.rows_per_core, cfg.n_win, cfg.split
    NCORE = cfg.n_cores

    src = np.asarray(edge_index[0], np.int64)
    dst = np.asarray(edge_index[1], np.int64)
    loops = np.arange(N, dtype=np.int64)
    src = np.concatenate([src, loops])
    dst = np.concatenate([dst, loops])

    core = dst // R
    ds = dst % R
    passB = (src >= SPLIT).astype(np.int64)
    win = ds // P

    cnt = np.zeros((NCORE, NWIN, 2), np.int64)
    np.add.at(cnt, (core, win, passB), 1)
    nchunk_w = (-(-cnt // P)).max(axis=0)  # [NWIN, 2] shared across cores
    nchunk_w = np.maximum(nchunk_w, [1, 0])

    groups = nchunk_w.reshape(-1)
    group_base = np.concatenate([[0], np.cumsum(groups)])
    total_chunks = int(group_base[-1])
    S = total_chunks * P

    win_first = np.array([group_base[2 * w] for w in range(NWIN)])
    win_nch = np.array([groups[2 * w] + groups[2 * w + 1] for w in range(NWIN)])

    # runs: (win, passB, chunk_start, n_chunks<=8)
    runs = []
    for w in range(NWIN):
        for pb in range(2):
            c0 = int(group_base[2 * w + pb])
            n = int(groups[2 * w + pb])
            while n > 0:
                k = min(8, n)
                runs.append((w, pb, c0, k))
                c0 += k
                n -= k

    gkey = win * 2 + passB
    per_core = []
    for c in range(NCORE):
        m = core == c
        ss, d, gk = src[m], ds[m], gkey[m]
        order = np.argsort(gk * R + d, kind="stable")
        ss, d, gk = ss[order], d[order], gk[order]
        gcnt = np.bincount(gk, minlength=2 * NWIN)
        gstart = np.concatenate([[0], np.cumsum(gcnt)])[:-1]
        within = np.arange(len(gk)) - np.repeat(gstart, gcnt)
        slot = group_base[gk] * P + within
        idx_t = np.zeros(S, np.int64)
        idx_d = np.zeros(S, np.int64)
        amat = np.zeros((P, total_chunks, P), np.float16)
        idx_t[slot] = np.where(gk % 2 == 0, ss, ss - SPLIT)
        idx_d[slot] = d
        amat[slot % P, slot // P, d % P] = 1.0
        per_core.append(dict(idxt=wrap_idx(idx_t), idxd=wrap_idx(idx_d),
                             amat=amat))

    return dict(total_chunks=total_chunks, runs=runs,
                win_first=[int(v) for v in win_first],
                win_nch=[int(v) for v in win_nch],
                per_core=per_core, S=S)


# ----------------------------------------------------------------------------
# Phase 1
# ----------------------------------------------------------------------------

def build_phase1(cfg: Cfg):
    key = ("p1", cfg.node_chunk)
    if key in _NC_CACHE:
        return _NC_CACHE[key]
    nc = bacc.Bacc("TRN2", target_bir_lowering=False)
    R = cfg.rows_per_core
    KT1 = cfg.d_in_pad // P   # 12
    KT2 = cfg.d_hid // P      # 8
    KT3 = cfg.d_out // P      # 4
    MT1 = cfg.d_hid // P      # 8
    MT2 = cfg.d_out // P      # 4
    W3 = cfg.w3               # 264
    RW = cfg.row_w            # 384
    NCK = cfg.node_chunk
    chunk_sizes = [NCK] * (R // NCK)
    if R % NCK:
        chunk_sizes.append(R % NCK)
    inv_din = 1.0 / cfg.d_in

    xT = nc.dram_tensor("xT", [cfg.d_in_pad, R], F16, kind="ExternalInput")
    W1p = nc.dram_tensor("W1p", [cfg.d_in_pad, cfg.d_hid], F16, kind="ExternalInput")
    W2 = nc.dram_tensor("W2", [cfg.d_hid, cfg.d_out], F16, kind="ExternalInput")
    Wgp = nc.dram_tensor("Wgp", [cfg.d_out, W3], F16, kind="ExternalInput")
    Wrp = nc.dram_tensor("Wrp", [cfg.d_out, W3], F16, kind="ExternalInput")
    w1s = nc.dram_tensor("w1s", [8, cfg.d_hid], F16, kind="ExternalInput")
    onep = nc.dram_tensor("onep", [8, P], F16, kind="ExternalInput")
    ones1 = nc.dram_tensor("ones1", [P, 1], F16, kind="ExternalInput")
    cvec = nc.dram_tensor("cvec", [P, MT1], F32, kind="ExternalInput")
    b2v = nc.dram_tensor("b2v", [P, MT2], F32, kind="ExternalInput")
    brpad = nc.dram_tensor("brpad", [P, W3], F32, kind="ExternalInput")

    gtab = nc.dram_tensor("gtab", [R, RW], F16, kind="ExternalOutput")
    ra = nc.dram_tensor("ra", [R, W3], F32, kind="ExternalOutput")

    with tile.TileContext(nc) as tc:
        with (
            tc.tile_pool(name="wpool", bufs=1) as wp,
            tc.tile_pool(name="xpool", bufs=2) as xp,
            tc.tile_pool(name="sqpool", bufs=2) as sqp,
            tc.tile_pool(name="hpool", bufs=2) as hp,
            tc.tile_pool(name="epool", bufs=3) as ep,
            tc.tile_pool(name="stat", bufs=2) as stp,
            tc.tile_pool(name="ps_y", bufs=2, space="PSUM") as ps_y,
            tc.tile_pool(name="ps_s", bufs=1, space="PSUM") as ps_s,
            tc.tile_pool(name="ps_o", bufs=2, space="PSUM") as ps_o,
        ):
            w1_sb = wp.tile([P, KT1, cfg.d_hid], F16)
            nc.sync.dma_start(w1_sb[:], W1p.rearrange("(kt p) m -> p kt m", p=P))
            w2_sb = wp.tile([P, KT2, cfg.d_out], F16)
            nc.sync.dma_start(w2_sb[:], W2.rearrange("(kt p) m -> p kt m", p=P))
            wg_sb = wp.tile([P, KT3, W3], F16)
            nc.sync.dma_start(wg_sb[:], Wgp.rearrange("(kt p) m -> p kt m", p=P))
            wr_sb = wp.tile([P, KT3, W3], F16)
            nc.sync.dma_start(wr_sb[:], Wrp.rearrange("(kt p) m -> p kt m", p=P))
            w1s_sb = wp.tile([8, cfg.d_hid], F16)
            nc.sync.dma_start(w1s_sb[:], w1s[:])
            onep_sb = wp.tile([8, P], F16)
            nc.sync.dma_start(onep_sb[:], onep[:])
            ones1_sb = wp.tile([P, 1], F16)
            nc.sync.dma_start(ones1_sb[:], ones1[:])
            cvec_sb = wp.tile([P, MT1], F32)
            nc.sync.dma_start(cvec_sb[:], cvec[:])
            b2_sb = wp.tile([P, MT2], F32)
            nc.sync.dma_start(b2_sb[:], b2v[:])
            brp_sb = wp.tile([P, W3], F32)
            nc.sync.dma_start(brp_sb[:], brpad[:])

            ns = 0
            for NC in chunk_sizes:
                xt = xp.tile([P, KT1, NC], F16, tag="xt")
                nc.sync.dma_start(
                    xt[:], xT.rearrange("(kt p) n -> p kt n", p=P)[:, :, ns:ns + NC])
                # stats
                s1_ps = ps_s.tile([1, NC], F32, tag="s1")
                for kt in range(KT1):
                    nc.tensor.matmul(s1_ps[:], ones1_sb[:], xt[:, kt],
                                     start=(kt == 0), stop=(kt == KT1 - 1))
                xsq = sqp.tile([P, KT1, NC], F16, tag="xsq")
                nc.vector.tensor_tensor(xsq[:], xt[:], xt[:], op=AL.mult)
                s2_ps = ps_s.tile([1, NC], F32, tag="s2")
                for kt in range(KT1):
                    nc.tensor.matmul(s2_ps[:], ones1_sb[:], xsq[:, kt],
                                     start=(kt == 0), stop=(kt == KT1 - 1))
                mu = stp.tile([1, NC], F32, tag="mu")
                nc.vector.tensor_scalar_mul(mu[:], s1_ps[:], inv_din)
                var = stp.tile([1, NC], F32, tag="var")
                nc.vector.tensor_scalar_mul(var[:], s2_ps[:], inv_din)
                musq = stp.tile([1, NC], F32, tag="musq")
                nc.vector.tensor_tensor(musq[:], mu[:], mu[:], op=AL.mult)
                nc.vector.tensor_tensor(var[:], var[:], musq[:], op=AL.subtract)
                nc.vector.tensor_scalar_add(var[:], var[:], 1e-5)
                sd = stp.tile([1, NC], F32, tag="sd")
                nc.scalar.activation(sd[:], var[:], AF.Sqrt)
                rstd = stp.tile([1, NC], F32, tag="rstd")
                nc.vector.reciprocal(rstd[:], sd[:])
                # [mur | rstd] rows for the two 8-partition matmuls
                mu_pad = stp.tile([8, NC], F16, tag="mup")
                nc.vector.memset(mu_pad[:], 0.0)
                nc.vector.tensor_tensor(mu_pad[0:1, :], mu[:], rstd[:], op=AL.mult)
                r_pad = stp.tile([8, NC], F16, tag="rp")
                nc.vector.memset(r_pad[:], 0.0)
                nc.vector.tensor_copy(r_pad[0:1, :], rstd[:])
                rb_ps = ps_s.tile([P, NC], F32, tag="rb")
                nc.tensor.matmul(rb_ps[:], onep_sb[:], r_pad[:], start=True, stop=True)
                rstd_b = stp.tile([P, NC], F16, tag="rstdb")
                nc.scalar.activation(rstd_b[:], rb_ps[:], AF.Identity)
                # x <- x * rstd (broadcast over kt)
                nc.vector.tensor_tensor(
                    xt[:], xt[:],
                    rstd_b[:].unsqueeze(1).to_broadcast([P, KT1, NC]), op=AL.mult)

                # L1
                h_sb = hp.tile([P, MT1, NC], F16, tag="h")
                for mt in range(MT1):
                    y_ps = ps_y.tile([P, NC], F32, tag="y")
                    for kt in range(KT1):
                        nc.tensor.matmul(y_ps[:], w1_sb[:, kt, mt * P:(mt + 1) * P],
                                         xt[:, kt], start=(kt == 0), stop=False)
                    nc.tensor.matmul(y_ps[:], w1s_sb[:, mt * P:(mt + 1) * P],
                                     mu_pad[:], start=False, stop=True)
                    nc.scalar.activation(h_sb[:, mt], y_ps[:], AF.Relu,
                                         bias=cvec_sb[:, mt:mt + 1])
                # L2
                h2_sb = hp.tile([P, MT2, NC], F16, tag="h2")
                for mt in range(MT2):
                    y2_ps = ps_y.tile([P, NC], F32, tag="y")
                    for kt in range(KT2):
                        nc.tensor.matmul(y2_ps[:], w2_sb[:, kt, mt * P:(mt + 1) * P],
                                         h_sb[:, kt], start=(kt == 0),
                                         stop=(kt == KT2 - 1))
                    nc.scalar.activation(h2_sb[:, mt], y2_ps[:], AF.Identity,
                                         bias=b2_sb[:, mt:mt + 1])
                # L3 row-major
                for nt in range(NC // P):
                    g_ps = ps_o.tile([P, W3], F32, tag="ops")
                    r_ps = ps_o.tile([P, W3], F32, tag="ops")
                    for kt in range(KT3):
                        nc.tensor.matmul(g_ps[:], h2_sb[:, kt, nt * P:(nt + 1) * P],
                                         wg_sb[:, kt], start=(kt == 0),
                                         stop=(kt == KT3 - 1))
                    for kt in range(KT3):
                        nc.tensor.matmul(r_ps[:], h2_sb[:, kt, nt * P:(nt + 1) * P],
                                         wr_sb[:, kt], start=(kt == 0),
                                         stop=(kt == KT3 - 1))
                    gt = ep.tile([P, W3], F16, tag="gt")
                    nc.scalar.activation(gt[:], g_ps[:], AF.Identity)
                    rt = ep.tile([P, W3], F32, tag="rt")
                    nc.vector.tensor_tensor(rt[:], r_ps[:], brp_sb[:], op=AL.add)
                    r0 = ns + nt * P
                    nc.sync.dma_start(gtab[r0:r0 + P, :W3], gt[:])
                    nc.sync.dma_start(ra[r0:r0 + P, :], rt[:])
                ns += NC
    nc.compile()
    _NC_CACHE[key] = nc
    return nc


# ----------------------------------------------------------------------------
# Phase 2
# ----------------------------------------------------------------------------

def build_phase2(cfg: Cfg, plan):
    key = ("p2", plan["total_chunks"], tuple(plan["runs"]))
    if key in _NC_CACHE:
        return _NC_CACHE[key]
    nc = bacc.Bacc("TRN2", target_bir_lowering=False)
    R = cfg.rows_per_core
    RW = cfg.row_w
    W3 = cfg.w3
    DH = cfg.d_head
    H = cfg.H
    C = cfg.C
    S = plan["S"]
    TC = plan["total_chunks"]
    NWIN = cfg.n_win
    runs = plan["runs"]
    win_first = plan["win_first"]
    win_nch = plan["win_nch"]
    win_last = [win_first[w] + win_nch[w] - 1 for w in range(NWIN)]

    tabA = nc.dram_tensor("tabA", [cfg.split, RW], F16, kind="ExternalInput")
    tabB = nc.dram_tensor("tabB", [cfg.table_rows - cfg.split, RW], F16,
                          kind="ExternalInput")
    adt = nc.dram_tensor("adt", [R, P], F16, kind="ExternalInput")
    resd = nc.dram_tensor("resd", [R, DH], F32, kind="ExternalInput")
    idxt = nc.dram_tensor("idxt", [P, S // 16], I16, kind="ExternalInput")
    idxd = nc.dram_tensor("idxd", [P, S // 16], I16, kind="ExternalInput")
    amat = nc.dram_tensor("amat", [P, TC, P], F16, kind="ExternalInput")
    bgc = nc.dram_tensor("bgc", [P, DH], F32, kind="ExternalInput")

    outp = nc.dram_tensor("outp", [R, DH], F32, kind="ExternalOutput")

    with tile.TileContext(nc) as tc:
        with (
            tc.tile_pool(name="const", bufs=1) as cp,
            tc.tile_pool(name="gath", bufs=3) as gp,
            tc.tile_pool(name="wk", bufs=3) as wk,
            tc.tile_pool(name="epi", bufs=2) as epp,
            tc.tile_pool(name="ps_w", bufs=2, space="PSUM") as psw,
        ):
            nc.gpsimd.load_library(mlp_lib)
            idxt_sb = cp.tile([P, S // 16], I16)
            nc.sync.dma_start(idxt_sb[:], idxt[:])
            idxd_sb = cp.tile([P, S // 16], I16)
            nc.sync.dma_start(idxd_sb[:], idxd[:])
            bg_sb = cp.tile([P, DH], F32)
            nc.sync.dma_start(bg_sb[:], bgc[:])

            win_ps = [None] * NWIN
            res_sb = [None] * NWIN

            for (w, pb, c0, nch) in runs:
                if c0 == win_first[w]:
                    win_ps[w] = psw.tile([P, W3], F32, tag="win",
                                         name=f"win{w}")
                    res_sb[w] = epp.tile([P, DH], F32, tag="res",
                                         name=f"res{w}")
                    nc.sync.dma_start(res_sb[w][:],
                                      resd[w * P:(w + 1) * P, :])
                ni = nch * P
                gt = gp.tile([P, 8, RW], F16, tag="gt", name=f"gt{c0}")
                tab = tabA if pb == 0 else tabB
                nc.gpsimd.dma_gather(
                    gt[:, :nch, :], tab[:],
                    idxt_sb[:, c0 * 8:(c0 + nch) * 8], ni, ni, RW)
                at = gp.tile([P, 8, P], F16, tag="at", name=f"at{c0}")
                nc.gpsimd.dma_gather(
                    at[:, :nch, :], adt[:],
                    idxd_sb[:, c0 * 8:(c0 + nch) * 8], ni, ni, P)
                am = wk.tile([P, 8, P], F16, tag="am", name=f"am{c0}")
                nc.sync.dma_start(am[:, :nch, :], amat[:, c0:c0 + nch, :])

                # e = lrelu(asrc + adst); ex = exp(e) -> msg[:, :, 256:264]
                msg = wk.tile([P, 8, W3], F16, tag="msg", name=f"m{c0}")
                et = wk.tile([P, 8, H], F16, tag="et", name=f"e{c0}")
                nc.vector.tensor_tensor(
                    et[:, :nch, :], gt[:, :nch, DH:DH + H],
                    at[:, :nch, :H], op=AL.add)
                nc.vector.scalar_tensor_tensor(et[:, :nch, :], et[:, :nch, :],
                                               0.2, et[:, :nch, :],
                                               op0=AL.mult, op1=AL.max)
                nc.scalar.activation(msg[:, :nch, DH:], et[:, :nch, :], AF.Exp)
                # msg[:, :, :256] = g * ex (broadcast over c, c-major)
                nc.vector.tensor_tensor(
                    msg[:, :nch, :DH].rearrange("p n (c h) -> p n c h", h=H),
                    gt[:, :nch, :DH].rearrange("p n (c h) -> p n c h", h=H),
                    msg[:, :nch, DH:].unsqueeze(2).to_broadcast([P, nch, C, H]),
                    op=AL.mult)

                for j in range(nch):
                    c = c0 + j
                    nc.tensor.matmul(win_ps[w][:], am[:, j, :], msg[:, j, :],
                                     start=(c == win_first[w]),
                                     stop=(c == win_last[w]))

                if c0 + nch - 1 == win_last[w]:
                    wt = win_ps[w]
                    dent = epp.tile([P, H], F32, tag="den")
                    nc.vector.tensor_scalar_add(dent[:], wt[:, DH:], 1e-16)
                    rec = epp.tile([P, H], F32, tag="rec")
                    nc.vector.reciprocal(rec[:], dent[:])
                    z = epp.tile([P, DH], F32, tag="z")
                    nc.vector.tensor_tensor(
                        z[:].rearrange("p (c h) -> p c h", h=H),
                        wt[:, :DH].rearrange("p (c h) -> p c h", h=H),
                        rec[:].unsqueeze(1).to_broadcast([P, C, H]),
                        op=AL.mult)
                    nc.vector.tensor_tensor(z[:], z[:], bg_sb[:], op=AL.add)
                    zm = epp.tile([P, DH], F32, tag="zm")
                    nc.vector.tensor_scalar_min(zm[:], z[:], 0.0)
                    ez = epp.tile([P, DH], F32, tag="ez")
                    nc.scalar.activation(ez[:], zm[:], AF.Exp)
                    o = epp.tile([P, DH], F32, tag="o")
                    nc.vector.scalar_tensor_tensor(o[:], z[:], 0.0, ez[:],
                                                   op0=AL.max, op1=AL.add)
                    nc.vector.scalar_tensor_tensor(o[:], o[:], -1.0,
                                                   res_sb[w][:],
                                                   op0=AL.add, op1=AL.add)
                    nc.sync.dma_start(outp[w * P:(w + 1) * P, :], o[:])
    nc.compile()
    _NC_CACHE[key] = nc
    return nc


# ----------------------------------------------------------------------------
# Host prep
# ----------------------------------------------------------------------------

def perm_ch(cfg: Cfg):
    """column permutation: c-major index (c*H + h) -> h-major (h*C + c)"""
    C, H = cfg.C, cfg.H
    p = np.zeros(C * H, np.int64)
    for c in range(C):
        for h in range(H):
            p[c * H + h] = h * C + c
    return p


def prep(cfg: Cfg, x, edge_index, ln_g, ln_b, W1, b1, W2, b2, Wr, br, Wg,
         att_src, att_dst, bg):
    N, R = cfg.n_nodes, cfg.rows_per_core
    NCORE = cfg.n_cores
    H, C = cfg.H, cfg.C

    x = np.asarray(x, np.float32)
    ln_g = np.asarray(ln_g, np.float32)
    ln_b = np.asarray(ln_b, np.float32)
    W1 = np.asarray(W1, np.float32)
    b1 = np.asarray(b1, np.float32)
    W2 = np.asarray(W2, np.float32)
    b2 = np.asarray(b2, np.float32)
    Wr = np.asarray(Wr, np.float32)
    br = np.asarray(br, np.float32)
    Wg = np.asarray(Wg, np.float32)
    att_src = np.asarray(att_src, np.float32)
    att_dst = np.asarray(att_dst, np.float32)
    bg = np.asarray(bg, np.float32)

    plan = build_plan(cfg, edge_index)

    pc = perm_ch(cfg)
    att_src_e = np.zeros((cfg.d_head, H), np.float32)
    att_dst_e = np.zeros((cfg.d_head, H), np.float32)
    for h in range(H):
        att_src_e[h * C:(h + 1) * C, h] = att_src[h]
        att_dst_e[h * C:(h + 1) * C, h] = att_dst[h]
    Wgp = np.concatenate([Wg[:, pc], Wg @ att_src_e], 1).astype(np.float16)
    Wrp = np.concatenate([Wr[:, pc], Wg @ att_dst_e], 1).astype(np.float16)

    W1p32 = W1 * ln_g[:, None]
    W1pad = np.zeros((cfg.d_in_pad, cfg.d_hid), np.float32)
    W1pad[:cfg.d_in] = W1p32
    w1s = np.zeros((8, cfg.d_hid), np.float32)
    w1s[0] = -W1pad.sum(axis=0)
    cvec = (b1 + ln_b @ W1).reshape(cfg.d_hid // P, P).T.astype(np.float32).copy()
    b2t = b2.reshape(cfg.d_out // P, P).T.astype(np.float32).copy()
    onep = np.zeros((8, P), np.float32)
    onep[0] = 1.0
    ones1 = np.ones((P, 1), np.float32)
    brpad = np.zeros((P, cfg.w3), np.float32)
    brpad[:, :cfg.d_head] = br[pc]
    bgc = np.tile(bg[pc].astype(np.float32), (P, 1))

    p1_shared = dict(
        W1p=W1pad.astype(np.float16), W2=W2.astype(np.float16),
        Wgp=Wgp, Wrp=Wrp, w1s=w1s.astype(np.float16),
        onep=onep.astype(np.float16), ones1=ones1.astype(np.float16),
        cvec=cvec, b2v=b2t, brpad=brpad)

    p1_maps = []
    for c in range(NCORE):
        n0 = c * R
        n1 = min((c + 1) * R, N)
        xs = np.zeros((R, cfg.d_in), np.float16)
        xs[:n1 - n0] = x[n0:n1].astype(np.float16)
        xt = np.zeros((cfg.d_in_pad, R), np.float16)
        xt[:cfg.d_in] = xs.T
        p1_maps.append(dict(xT=xt, **p1_shared))

    return p1_maps, plan, dict(bgc=bgc, pc=pc)


def kernel(**inputs) -> np.ndarray:
    cfg = CFG
    N, R = cfg.n_nodes, cfg.rows_per_core
    NCORE = cfg.n_cores
    DH = cfg.d_head

    p1_maps, plan, meta = prep(cfg, **inputs)

    nc1 = build_phase1(cfg)
    r1 = run_bass_kernel_spmd(nc1, p1_maps, core_ids=list(range(NCORE)))
    gtab_full = np.concatenate([r1.results[c]["gtab"] for c in range(NCORE)], 0)
    ras = [r1.results[c]["ra"] for c in range(NCORE)]

    nc2 = build_phase2(cfg, plan)
    p2_maps = []
    tabA = gtab_full[:cfg.split]
    tabB = gtab_full[cfg.split:]
    for c in range(NCORE):
        adt = np.zeros((R, P), np.float16)
        adt[:, :cfg.H] = ras[c][:, DH:].astype(np.float16)
        p2_maps.append(dict(
            tabA=tabA, tabB=tabB, adt=adt,
            resd=np.ascontiguousarray(ras[c][:, :DH]),
            idxt=plan["per_core"][c]["idxt"], idxd=plan["per_core"][c]["idxd"],
            amat=plan["per_core"][c]["amat"], bgc=meta["bgc"]))
    r2 = run_bass_kernel_spmd(nc2, p2_maps, core_ids=list(range(NCORE)))

    outc = np.concatenate([r2.results[c]["outp"] for c in range(NCORE)], 0)
    out = np.empty((N, DH), np.float32)
    out[:, meta["pc"]] = outc[:N]
    return out
